# revision 3
# baseline (speedup 1.0000x reference)
"""Block-sparse linear kernel for Trainium2 (8 NeuronCores, data-parallel).

Computes out = 2 * (x @ (weight*mask).T) + bias for
x: (8, 2048, 4096) f32, weight: (4096, 4096) f32, bias: (4096,) f32,
block_mask: (128, 128) bool over 32x32 blocks.

Strategy: shard x on batch across the 8 cores (weight/bias replicated).
The mask and the *2 scale are folded into the weight on the host, so each
core runs a dense M=2048, K=4096, N=4096 GEMM in bf16 with fp32 PSUM
accumulation:
  - x.T tiles (contraction dim on partitions) stay resident in SBUF (16 MiB)
  - weight.T streams through in o-chunks, double buffered
  - bias is added during the PSUM->SBUF eviction on the vector engine
"""
import os

import numpy as np

# Problem constants (hardcoded per the harness contract).
B, S, IN, OUT = 8, 2048, 4096, 4096
BLOCK = 32
P = 128                    # partitions / contraction tile
IT = IN // P               # 32 i-tiles
OC = 256                   # o-chunk width (matmul free dim)
NOC = OUT // OC            # 16 o-chunks
ST = S // P                # 16 s-tiles

LAST_EXEC_NS = None


def _build_program():
    import concourse.bacc as bacc
    import concourse.tile as tile
    from concourse import mybir

    nc = bacc.Bacc("TRN2", debug=False, num_devices=B)
    x_d = nc.dram_tensor("xt", (IT, P, S), mybir.dt.bfloat16, kind="ExternalInput")
    w_d = nc.dram_tensor("wt", (NOC, P, IT, OC), mybir.dt.bfloat16, kind="ExternalInput")
    b_d = nc.dram_tensor("bias", (NOC, P, OC), mybir.dt.float32, kind="ExternalInput")
    o_d = nc.dram_tensor("out", (S, OUT), mybir.dt.float32, kind="ExternalOutput")

    with tile.TileContext(nc) as tc:
        with (
            tc.tile_pool(name="xpool", bufs=1) as xp,
            tc.tile_pool(name="wpool", bufs=2) as wp,
            tc.tile_pool(name="bpool", bufs=2) as bp,
            tc.tile_pool(name="opool", bufs=4) as op,
            tc.tile_pool(name="psum", bufs=4, space="PSUM") as pp,
        ):
            xt = []
            for it in range(IT):
                t = xp.tile([P, S], mybir.dt.bfloat16, tag=f"x{it}")
                nc.sync.dma_start(out=t[:], in_=x_d[it])
                xt.append(t)
            for oc in range(NOC):
                wc = wp.tile([P, IT, OC], mybir.dt.bfloat16, tag="w")
                nc.sync.dma_start(out=wc[:], in_=w_d[oc])
                bt = bp.tile([P, OC], mybir.dt.float32, tag="b")
                nc.sync.dma_start(out=bt[:], in_=b_d[oc])
                for st in range(ST):
                    ps = pp.tile([P, OC], mybir.dt.float32, tag="ps")
                    for it in range(IT):
                        nc.tensor.matmul(
                            ps[:],
                            xt[it][:, st * P:(st + 1) * P],
                            wc[:, it, :],
                            start=(it == 0),
                            stop=(it == IT - 1),
                        )
                    ot = op.tile([P, OC], mybir.dt.float32, tag="o")
                    nc.vector.tensor_add(out=ot[:], in0=ps[:], in1=bt[:])
                    nc.sync.dma_start(
                        out=o_d[st * P:(st + 1) * P, oc * OC:(oc + 1) * OC],
                        in_=ot[:],
                    )
    nc.compile()
    return nc


def _install_axon_ntff_hook(so_path="/opt/axon/libaxon_pjrt.so"):
    """Make run_bass_kernel_spmd(trace=True) work when the image's antenv
    lacks axon_hooks: drive NTFF profiling via ctypes on libaxon_pjrt.so."""
    import contextlib
    import ctypes
    import sys
    import types

    lib = ctypes.CDLL(so_path)
    if not hasattr(lib, "axon_start_nrt_profile"):
        return
    lib.axon_start_nrt_profile.argtypes = [
        ctypes.POINTER(ctypes.c_int64),
        ctypes.c_size_t,
    ]
    lib.axon_start_nrt_profile.restype = ctypes.c_int64
    lib.axon_stop_nrt_profile.argtypes = [ctypes.c_char_p]
    lib.axon_stop_nrt_profile.restype = ctypes.c_int64

    @contextlib.contextmanager
    def _hook(output_dir, device_ids):
        import jax

        jax.devices()
        if device_ids:
            ids = (ctypes.c_int64 * len(device_ids))(*device_ids)
            rc = lib.axon_start_nrt_profile(ids, len(device_ids))
        else:
            rc = lib.axon_start_nrt_profile(None, 0)
        if rc != 0:
            raise RuntimeError(f"axon_start_nrt_profile rc={rc}")
        try:
            yield
        finally:
            n = lib.axon_stop_nrt_profile(str(output_dir).encode())
            print(f"ntff profile: {n} file(s) -> {output_dir}", file=sys.stderr)

    mod = types.ModuleType("antenv.axon_hooks")
    mod.get_axon_ntff_profile_hook = lambda: _hook
    mod.set_axon_ntff_profile_hook = lambda h: None
    sys.modules["antenv.axon_hooks"] = mod

    import concourse.bass_utils as bu

    bu.upload_artifacts = lambda tmpdir: f"file://{tmpdir}"


def kernel(x, weight, bias, block_mask):
    global LAST_EXEC_NS
    from concourse.bass_utils import run_bass_kernel_spmd
    from concourse import mybir

    bf16 = mybir.dt.np(mybir.dt.bfloat16)

    # Host-side prep: fold mask and the x2 into the weight, pre-transpose.
    mask = np.repeat(np.repeat(np.asarray(block_mask), BLOCK, 0), BLOCK, 1)
    w_eff = (2.0 * np.asarray(weight, np.float32)) * mask
    wt = np.ascontiguousarray(w_eff.T)                       # [IN, OUT]
    # [NOC, P, IT, OC]: per o-chunk, partition = i % 128 ... i laid out so
    # SBUF tile [P, IT, OC] has element (p, it, o) = wt[it*P+p, oc*OC+o].
    w_dev = np.ascontiguousarray(
        wt.reshape(IT, P, NOC, OC).transpose(2, 1, 0, 3)
    ).astype(bf16)
    b_dev = np.ascontiguousarray(
        np.broadcast_to(
            np.asarray(bias, np.float32).reshape(NOC, 1, OC), (NOC, P, OC)
        )
    )

    xs = np.asarray(x, np.float32)
    in_maps = []
    for b in range(B):
        x_dev = np.ascontiguousarray(xs[b].T).astype(bf16).reshape(IT, P, S)
        in_maps.append({"xt": x_dev, "wt": w_dev, "bias": b_dev})

    nc = _build_program()
    trace = bool(int(os.environ.get("BSL_TRACE", "0")))
    if trace:
        _install_axon_ntff_hook()
    res = run_bass_kernel_spmd(
        nc, in_maps, list(range(B)), trace=trace,
    )
    LAST_EXEC_NS = res.exec_time_ns
    return np.stack([res.results[b]["out"] for b in range(B)]).astype(np.float32)
